# revision 11
# baseline (speedup 1.0000x reference)
"""Bahdanau additive attention on 8 Trainium2 NeuronCores.

Data-parallel over batch B=64 (8 examples/core). Per example:
  encoded^T = W_enc^T @ enc^T   (f32r matmuls; enc transposed on-chip via PE)
  result    = tanh(encoded^T + decoded^T)  (ACT, fp16 out)
  logits    = W_out^T @ result  (fp16 matmuls -> [1,512] rows -> [128,32] layout)
  probs     = softmax(mask(logits))        (DVE/ACT + PE cross-partition reduce)
  attn      = probs @ enc                  (f32r matmuls on natural enc tiles)

Precision: main matmuls run in float32r (fp32 bits, PE rounds to ~11 mantissa
bits on read) -> probs/attn scale-relative error ~6e-3 vs fp32 reference.
"""
import numpy as np

import concourse.bass as bass
import concourse.mybir as mybir
from concourse import bacc, bass_utils
from concourse.tile import TileContext

N_CORES = 8
B, S, E, H = 64, 4096, 512, 512
BPC = B // N_CORES            # examples per core
ST = S // 512                 # 8 s-tiles of 512 per example
f32 = mybir.dt.float32
f32r = mybir.dt.float32r
f16 = mybir.dt.float16
AX = mybir.AxisListType.X
AF = mybir.ActivationFunctionType

_built = {}


def _build():
    if "nc" in _built:
        return _built["nc"]
    nc = bacc.Bacc("TRN2", target_bir_lowering=False, debug=False,
                   num_devices=N_CORES)
    enc_d = nc.dram_tensor("enc", [BPC, S, E], f32r, kind="ExternalInput")
    decT_d = nc.dram_tensor("decT", [4, 128, BPC], f32, kind="ExternalInput")
    madd_d = nc.dram_tensor("madd", [BPC, 128, S // 128], f32,
                            kind="ExternalInput")
    wenc_d = nc.dram_tensor("wenc", [E, H], f32r, kind="ExternalInput")
    wdec_d = nc.dram_tensor("wdec", [E, H], f32, kind="ExternalInput")
    wout_d = nc.dram_tensor("wout16", [4, 128, 1], f32r, kind="ExternalInput")
    id32_d = nc.dram_tensor("id32", [128, 128], f32, kind="ExternalInput")
    idr_d = nc.dram_tensor("idr", [128, 128], f32r, kind="ExternalInput")
    ones_d = nc.dram_tensor("ones", [128, 1], f32, kind="ExternalInput")
    attn_d = nc.dram_tensor("attn_out", [BPC, E], f32, kind="ExternalOutput")
    probs_d = nc.dram_tensor("probs_out", [BPC, S], f32, kind="ExternalOutput")

    with TileContext(nc) as tc:
        with (tc.tile_pool(name="const", bufs=1) as cpool,
              tc.tile_pool(name="encN", bufs=16) as encN_pool,
              tc.tile_pool(name="encT", bufs=9) as encT_pool,
              tc.tile_pool(name="res", bufs=8) as res_pool,
              tc.tile_pool(name="small", bufs=4) as small_pool,
              tc.tile_pool(name="expool", bufs=3) as ex_pool,
              tc.tile_pool(name="psT", bufs=2, space="PSUM") as psT,
              tc.tile_pool(name="psM", bufs=3, space="PSUM") as psM,
              tc.tile_pool(name="psS", bufs=2, space="PSUM") as psS,
              tc.tile_pool(name="psA", bufs=1, space="PSUM") as psA):
            # ---- constants (dec-path + identity first so PE starts early) ----
            decT_sb0 = []
            for db in range(4):
                t = cpool.tile([128, BPC], f32, name=f"decT{db}")
                nc.sync.dma_start(t[:], decT_d.ap()[db])
                decT_sb0.append(t)
            wdec_sb0 = []
            for db in range(4):
                t = cpool.tile([128, H], f32, name=f"wdec{db}")
                nc.sync.dma_start(t[:], wdec_d.ap()[128 * db:128 * (db + 1), :])
                wdec_sb0.append(t)
            idr = cpool.tile([128, 128], f32r, name="idrsb")
            nc.sync.dma_start(idr[:], idr_d.ap())
            id32 = cpool.tile([128, 128], f32, name="id32sb")
            nc.sync.dma_start(id32[:], id32_d.ap())
            wenc_sb = []
            for eb in range(4):
                t = cpool.tile([128, H], f32r, name=f"wenc{eb}")
                nc.sync.dma_start(t[:], wenc_d.ap()[128 * eb:128 * (eb + 1), :])
                wenc_sb.append(t)
            ones_sb = cpool.tile([128, 1], f32, name="ones_sb")
            nc.sync.dma_start(ones_sb[:], ones_d.ap())
            wout16 = cpool.tile([128, 4], f32r, name="wout16sb")
            for hb in range(4):
                nc.sync.dma_start(wout16[:, hb:hb + 1], wout_d.ap()[hb])

            # ---- per-example pipeline ----
            # ---- decoded^T = W_dec^T @ dec^T for all BPC examples ----
            decT_sb = decT_sb0
            wdec_sb = wdec_sb0
            dec_t = []
            for hb in range(4):
                pd = psM.tile([128, BPC], f32, tag="psM", name=f"pdec{hb}")
                for db in range(4):
                    nc.tensor.matmul(pd[:], wdec_sb[db][:, 128 * hb:128 * (hb + 1)],
                                     decT_sb[db][:], start=(db == 0),
                                     stop=(db == 3))
                t = cpool.tile([128, BPC], f32, name=f"decthb{hb}")
                nc.vector.tensor_copy(t[:], pd[:])
                dec_t.append(t)

            for ex in range(BPC):
                chunks = []
                for st in range(ST):
                    ch = encN_pool.tile([128, 2048], f32r, tag="encN",
                                        name=f"encN_{ex}_{st}")
                    src = enc_d.ap()[ex].rearrange(
                        "(st j p) e -> st p j e", j=4, p=128)[st]
                    nc.sync.dma_start(
                        ch[:].rearrange("p (j e) -> p j e", j=4), src)
                    chunks.append(ch)
                madd_t = ex_pool.tile([128, S // 128], f32, tag="madd",
                                      name=f"madd{ex}")
                nc.sync.dma_start(madd_t[:], madd_d.ap()[ex])

                logits32 = ex_pool.tile([128, S // 128], f32, tag="logits",
                                        name=f"lg{ex}")
                for sp in range(ST // 2):
                    sts = (2 * sp, 2 * sp + 1)
                    # transpose enc chunks -> encT tiles [128e, 512s]
                    encT = {}
                    for st in sts:
                        for eb in range(4):
                            pT = psT.tile([128, 512], f32r, tag="psT",
                                          name=f"pT{ex}_{st}_{eb}")
                            for j in range(4):
                                o = j * 512 + eb * 128
                                nc.tensor.transpose(
                                    pT[:, 128 * j:128 * (j + 1)],
                                    chunks[st][:, o:o + 128], idr[:])
                            eT = encT_pool.tile([128, 512], f32r, tag="encT",
                                                name=f"eT{ex}_{st}_{eb}")
                            if eb % 4 == 0:
                                nc.vector.tensor_copy(eT[:], pT[:])
                            else:
                                nc.scalar.copy(eT[:], pT[:])
                            encT[(st, eb)] = eT
                    # encoded^T + tanh -> fp16 result tiles [128h, 512s].
                    # Both s-tiles of the pair share each stationary W block
                    # (one LDWEIGHTS amortized over two matmuls).
                    res16 = {}
                    for hb in range(4):
                        pM = {st: psM.tile([128, 512], f32, tag="psM",
                                           name=f"pM{ex}_{st}_{hb}")
                              for st in sts}
                        for eb in range(4):
                            for st in sts:
                                nc.tensor.matmul(
                                    pM[st][:],
                                    wenc_sb[eb][:, 128 * hb:128 * (hb + 1)],
                                    encT[(st, eb)][:], start=(eb == 0),
                                    stop=(eb == 3))
                        for st in sts:
                            r = res_pool.tile([128, 512], f32r, tag="res",
                                              name=f"res{ex}_{st}_{hb}")
                            nc.scalar.activation(r[:], pM[st][:], AF.Tanh,
                                                 bias=dec_t[hb][:, ex:ex + 1])
                            res16[(st, hb)] = r
                    for st in sts:
                        # logits row [1, 512]
                        pL = psS.tile([1, 512], f32, tag="psS",
                                      name=f"pL{ex}_{st}")
                        for hb in range(4):
                            nc.tensor.matmul(pL[:], wout16[:, hb:hb + 1],
                                             res16[(st, hb)][:],
                                             start=(hb == 0), stop=(hb == 3))
                        lrow = small_pool.tile([1, 512], f32, tag="lrow",
                                               name=f"lrow{ex}_{st}")
                        nc.vector.tensor_copy(lrow[:], pL[:])
                        # [1,512] -> [128,4] via K=1 PE transposes
                        pLT = psS.tile([128, 4], f32, tag="psS",
                                       name=f"pLT{ex}_{st}")
                        for j in range(4):
                            nc.tensor.transpose(
                                pLT[:, j:j + 1],
                                lrow[0:1, 128 * j:128 * (j + 1)],
                                id32[0:1, 0:1])
                        nc.vector.tensor_copy(logits32[:, 4 * st:4 * st + 4],
                                              pLT[:])

                # ---- softmax over the 4096 logits ----
                masked = ex_pool.tile([128, S // 128], f32, tag="masked",
                                      name=f"msk{ex}")
                nc.vector.tensor_add(masked[:], logits32[:], madd_t[:])
                mx128 = ex_pool.tile([128, 1], f32, tag="mx", name=f"mx{ex}")
                nc.vector.reduce_max(mx128[:], masked[:], axis=AX)
                pmx = psS.tile([1, 128], f32, tag="psS", name=f"pmx{ex}")
                nc.tensor.transpose(pmx[:], mx128[:], id32[:])
                mxrow = small_pool.tile([1, 128], f32, tag="mxrow",
                                        name=f"mxrow{ex}")
                nc.vector.tensor_copy(mxrow[:], pmx[:])
                mxs = small_pool.tile([1, 1], f32, tag="mxs", name=f"mxs{ex}")
                nc.vector.reduce_max(mxs[:], mxrow[:], axis=AX)
                negM = small_pool.tile([1, 1], f32, tag="negM",
                                       name=f"negM{ex}")
                nc.vector.tensor_scalar_mul(negM[:], mxs[:], -1.0)
                negMb = ex_pool.tile([128, 1], f32, tag="negMb",
                                     name=f"negMb{ex}")
                nc.gpsimd.partition_broadcast(negMb[:], negM[:])
                exp32 = ex_pool.tile([128, S // 128], f32, tag="exp",
                                     name=f"exp{ex}")
                nc.scalar.activation(exp32[:], masked[:], AF.Exp,
                                     bias=negMb[:])
                s128 = ex_pool.tile([128, 1], f32, tag="s128",
                                    name=f"s128{ex}")
                nc.vector.reduce_sum(s128[:], exp32[:], axis=AX)
                ps_sum = psS.tile([1, 1], f32, tag="psS", name=f"pssum{ex}")
                nc.tensor.matmul(ps_sum[:], s128[:], ones_sb[:], start=True,
                                 stop=True)
                rs = small_pool.tile([1, 1], f32, tag="rs", name=f"rs{ex}")
                nc.vector.reciprocal(rs[:], ps_sum[:])
                rsb = ex_pool.tile([128, 1], f32, tag="rsb", name=f"rsb{ex}")
                nc.gpsimd.partition_broadcast(rsb[:], rs[:])
                probs_t = ex_pool.tile([128, S // 128], f32r, tag="probs",
                                       name=f"probs{ex}")
                nc.vector.tensor_scalar_mul(probs_t[:], exp32[:], rsb[:])
                pPT = psS.tile([32, 128], f32r, tag="psS",
                               name=f"pPT{ex}")
                nc.tensor.transpose(pPT[:], probs_t[:], idr[:])
                probs_row = small_pool.tile([32, 128], f32r, tag="prow",
                                            name=f"prow{ex}")
                nc.vector.tensor_copy(probs_row[:], pPT[:])
                nc.sync.dma_start(
                    probs_d.ap()[ex].rearrange("(j p) -> j p", p=128),
                    probs_row[:].bitcast(f32))

                # ---- attn = probs @ enc (accumulate over 32 s-blocks) ----
                pA = psA.tile([1, 512], f32, tag="psA", name=f"pA{ex}")
                for st in range(ST):
                    for j in range(4):
                        c = 4 * st + j
                        nc.tensor.matmul(pA[:], probs_t[:, c:c + 1],
                                         chunks[st][:, 512 * j:512 * (j + 1)],
                                         start=(c == 0), stop=(c == 31))
                arow = small_pool.tile([1, 512], f32, tag="arow",
                                       name=f"arow{ex}")
                nc.vector.tensor_copy(arow[:], pA[:])
                nc.sync.dma_start(attn_d.ap()[ex:ex + 1, :], arow[:])
    nc.compile()
    _built["nc"] = nc
    return nc


def kernel(enc, dec, inp_mask, W_enc, W_dec, W_out, _trace=False):
    enc = np.ascontiguousarray(np.asarray(enc, dtype=np.float32))
    dec = np.asarray(dec, dtype=np.float32)
    inp_mask = np.asarray(inp_mask)
    W_enc = np.ascontiguousarray(np.asarray(W_enc, dtype=np.float32))
    W_dec = np.ascontiguousarray(np.asarray(W_dec, dtype=np.float32))
    W_out = np.asarray(W_out, dtype=np.float32)

    nc = _build()
    madd = (inp_mask.astype(np.float32) - 1.0) * 1.0e9          # [B, S]
    madd = np.ascontiguousarray(
        madd.reshape(B, S // 128, 128).transpose(0, 2, 1))       # [B,128,S/128]
    wout16 = np.ascontiguousarray(W_out.reshape(4, 128, 1))
    id32 = np.eye(128, dtype=np.float32)
    ones = np.ones((128, 1), dtype=np.float32)

    in_maps = []
    for c in range(N_CORES):
        b0 = c * BPC
        decT = np.ascontiguousarray(
            dec[b0:b0 + BPC].T.reshape(4, 128, BPC))
        in_maps.append({
            "enc": enc[b0:b0 + BPC],
            "decT": decT,
            "madd": madd[b0:b0 + BPC],
            "wenc": W_enc,
            "wdec": W_dec,
            "wout16": wout16,
            "id32": id32,
            "idr": id32,
            "ones": ones,
        })
    res = bass_utils.run_bass_kernel_spmd(
        nc, in_maps, core_ids=list(range(N_CORES)), trace=_trace)
    attn = np.concatenate([res.results[c]["attn_out"] for c in range(N_CORES)])
    probs = np.concatenate([res.results[c]["probs_out"]
                            for c in range(N_CORES)])
    if _trace:
        kernel._last_exec_time_ns = res.exec_time_ns
    return (attn.astype(np.float32), probs.astype(np.float32))


# revision 12
# speedup vs baseline: 1.0031x; 1.0031x over previous
"""Bahdanau additive attention on 8 Trainium2 NeuronCores.

Data-parallel over batch B=64 (8 examples/core). Per example:
  encoded^T = W_enc^T @ enc^T   (f32r matmuls; enc transposed on-chip via PE)
  result    = tanh(encoded^T + decoded^T)  (ACT, fp16 out)
  logits    = W_out^T @ result  (fp16 matmuls -> [1,512] rows -> [128,32] layout)
  probs     = softmax(mask(logits))        (DVE/ACT + PE cross-partition reduce)
  attn      = probs @ enc                  (f32r matmuls on natural enc tiles)

Precision: main matmuls run in float32r (fp32 bits, PE rounds to ~11 mantissa
bits on read) -> probs/attn scale-relative error ~6e-3 vs fp32 reference.
"""
import numpy as np

import concourse.bass as bass
import concourse.mybir as mybir
from concourse import bacc, bass_utils
from concourse.tile import TileContext

N_CORES = 8
B, S, E, H = 64, 4096, 512, 512
BPC = B // N_CORES            # examples per core
ST = S // 512                 # 8 s-tiles of 512 per example
f32 = mybir.dt.float32
f32r = mybir.dt.float32r
f16 = mybir.dt.float16
AX = mybir.AxisListType.X
AF = mybir.ActivationFunctionType

_built = {}


def _build():
    if "nc" in _built:
        return _built["nc"]
    nc = bacc.Bacc("TRN2", target_bir_lowering=False, debug=False,
                   num_devices=N_CORES)
    enc_d = nc.dram_tensor("enc", [BPC, S, E], f32r, kind="ExternalInput")
    decT_d = nc.dram_tensor("decT", [4, 128, BPC], f32, kind="ExternalInput")
    madd_d = nc.dram_tensor("madd", [BPC, 128, S // 128], f32,
                            kind="ExternalInput")
    wenc_d = nc.dram_tensor("wenc", [E, H], f32r, kind="ExternalInput")
    wdec_d = nc.dram_tensor("wdec", [E, H], f32, kind="ExternalInput")
    wout_d = nc.dram_tensor("wout16", [4, 128, 1], f32r, kind="ExternalInput")
    id32_d = nc.dram_tensor("id32", [128, 128], f32, kind="ExternalInput")
    idr_d = nc.dram_tensor("idr", [128, 128], f32r, kind="ExternalInput")
    ones_d = nc.dram_tensor("ones", [128, 1], f32, kind="ExternalInput")
    attn_d = nc.dram_tensor("attn_out", [BPC, E], f32, kind="ExternalOutput")
    probs_d = nc.dram_tensor("probs_out", [BPC, S], f32, kind="ExternalOutput")

    with TileContext(nc) as tc:
        with (tc.tile_pool(name="const", bufs=1) as cpool,
              tc.tile_pool(name="encN", bufs=16) as encN_pool,
              tc.tile_pool(name="encT", bufs=9) as encT_pool,
              tc.tile_pool(name="res", bufs=8) as res_pool,
              tc.tile_pool(name="small", bufs=4) as small_pool,
              tc.tile_pool(name="expool", bufs=3) as ex_pool,
              tc.tile_pool(name="psT", bufs=2, space="PSUM") as psT,
              tc.tile_pool(name="psM", bufs=3, space="PSUM") as psM,
              tc.tile_pool(name="psS", bufs=2, space="PSUM") as psS,
              tc.tile_pool(name="psA", bufs=1, space="PSUM") as psA):
            # ---- constants (dec-path + identity first so PE starts early) ----
            decT_sb0 = []
            for db in range(4):
                t = cpool.tile([128, BPC], f32, name=f"decT{db}")
                nc.sync.dma_start(t[:], decT_d.ap()[db])
                decT_sb0.append(t)
            wdec_sb0 = []
            for db in range(4):
                t = cpool.tile([128, H], f32, name=f"wdec{db}")
                nc.sync.dma_start(t[:], wdec_d.ap()[128 * db:128 * (db + 1), :])
                wdec_sb0.append(t)
            idr = cpool.tile([128, 128], f32r, name="idrsb")
            nc.sync.dma_start(idr[:], idr_d.ap())
            id32 = cpool.tile([128, 128], f32, name="id32sb")
            nc.sync.dma_start(id32[:], id32_d.ap())
            wenc_sb = []
            for eb in range(4):
                t = cpool.tile([128, H], f32r, name=f"wenc{eb}")
                nc.sync.dma_start(t[:], wenc_d.ap()[128 * eb:128 * (eb + 1), :])
                wenc_sb.append(t)
            ones_sb = cpool.tile([128, 1], f32, name="ones_sb")
            nc.sync.dma_start(ones_sb[:], ones_d.ap())
            wout16 = cpool.tile([128, 4], f32r, name="wout16sb")
            for hb in range(4):
                nc.sync.dma_start(wout16[:, hb:hb + 1], wout_d.ap()[hb])

            # ---- per-example pipeline ----
            # ---- decoded^T = W_dec^T @ dec^T for all BPC examples ----
            decT_sb = decT_sb0
            wdec_sb = wdec_sb0
            dec_t = []
            for hb in range(4):
                pd = psM.tile([128, BPC], f32, tag="psM", name=f"pdec{hb}")
                for db in range(4):
                    nc.tensor.matmul(pd[:], wdec_sb[db][:, 128 * hb:128 * (hb + 1)],
                                     decT_sb[db][:], start=(db == 0),
                                     stop=(db == 3))
                t = cpool.tile([128, BPC], f32, name=f"decthb{hb}")
                nc.vector.tensor_copy(t[:], pd[:])
                dec_t.append(t)

            for ex in range(BPC):
                chunks = []
                for st in range(ST):
                    ch = encN_pool.tile([128, 2048], f32r, tag="encN",
                                        name=f"encN_{ex}_{st}")
                    src = enc_d.ap()[ex].rearrange(
                        "(st j p) e -> st p j e", j=4, p=128)[st]
                    nc.sync.dma_start(
                        ch[:].rearrange("p (j e) -> p j e", j=4), src)
                    chunks.append(ch)
                madd_t = ex_pool.tile([128, S // 128], f32, tag="madd",
                                      name=f"madd{ex}")
                nc.sync.dma_start(madd_t[:], madd_d.ap()[ex])

                logits32 = ex_pool.tile([128, S // 128], f32, tag="logits",
                                        name=f"lg{ex}")
                for sp in range(ST // 2):
                    sts = (2 * sp, 2 * sp + 1)
                    # transpose enc chunks -> encT tiles [128e, 512s]
                    encT = {}
                    for st in sts:
                        for eb in range(4):
                            pT = psT.tile([128, 512], f32r, tag="psT",
                                          name=f"pT{ex}_{st}_{eb}")
                            for j in range(4):
                                o = j * 512 + eb * 128
                                nc.tensor.transpose(
                                    pT[:, 128 * j:128 * (j + 1)],
                                    chunks[st][:, o:o + 128], idr[:])
                            eT = encT_pool.tile([128, 512], f32r, tag="encT",
                                                name=f"eT{ex}_{st}_{eb}")
                            if eb % 2 == 0:
                                nc.vector.tensor_copy(eT[:], pT[:])
                            else:
                                nc.scalar.copy(eT[:], pT[:])
                            encT[(st, eb)] = eT
                    # encoded^T + tanh -> fp16 result tiles [128h, 512s].
                    # Both s-tiles of the pair share each stationary W block
                    # (one LDWEIGHTS amortized over two matmuls).
                    res16 = {}
                    for hb in range(4):
                        pM = {st: psM.tile([128, 512], f32, tag="psM",
                                           name=f"pM{ex}_{st}_{hb}")
                              for st in sts}
                        for eb in range(4):
                            for st in sts:
                                nc.tensor.matmul(
                                    pM[st][:],
                                    wenc_sb[eb][:, 128 * hb:128 * (hb + 1)],
                                    encT[(st, eb)][:], start=(eb == 0),
                                    stop=(eb == 3))
                        for st in sts:
                            r = res_pool.tile([128, 512], f32r, tag="res",
                                              name=f"res{ex}_{st}_{hb}")
                            nc.scalar.activation(r[:], pM[st][:], AF.Tanh,
                                                 bias=dec_t[hb][:, ex:ex + 1])
                            res16[(st, hb)] = r
                    for st in sts:
                        # logits row [1, 512]
                        pL = psS.tile([1, 512], f32, tag="psS",
                                      name=f"pL{ex}_{st}")
                        for hb in range(4):
                            nc.tensor.matmul(pL[:], wout16[:, hb:hb + 1],
                                             res16[(st, hb)][:],
                                             start=(hb == 0), stop=(hb == 3))
                        lrow = small_pool.tile([1, 512], f32, tag="lrow",
                                               name=f"lrow{ex}_{st}")
                        nc.vector.tensor_copy(lrow[:], pL[:])
                        # [1,512] -> [128,4] via K=1 PE transposes
                        pLT = psS.tile([128, 4], f32, tag="psS",
                                       name=f"pLT{ex}_{st}")
                        for j in range(4):
                            nc.tensor.transpose(
                                pLT[:, j:j + 1],
                                lrow[0:1, 128 * j:128 * (j + 1)],
                                id32[0:1, 0:1])
                        nc.vector.tensor_copy(logits32[:, 4 * st:4 * st + 4],
                                              pLT[:])

                # ---- softmax over the 4096 logits ----
                masked = ex_pool.tile([128, S // 128], f32, tag="masked",
                                      name=f"msk{ex}")
                nc.vector.tensor_add(masked[:], logits32[:], madd_t[:])
                mx128 = ex_pool.tile([128, 1], f32, tag="mx", name=f"mx{ex}")
                nc.vector.reduce_max(mx128[:], masked[:], axis=AX)
                pmx = psS.tile([1, 128], f32, tag="psS", name=f"pmx{ex}")
                nc.tensor.transpose(pmx[:], mx128[:], id32[:])
                mxrow = small_pool.tile([1, 128], f32, tag="mxrow",
                                        name=f"mxrow{ex}")
                nc.vector.tensor_copy(mxrow[:], pmx[:])
                mxs = small_pool.tile([1, 1], f32, tag="mxs", name=f"mxs{ex}")
                nc.vector.reduce_max(mxs[:], mxrow[:], axis=AX)
                negM = small_pool.tile([1, 1], f32, tag="negM",
                                       name=f"negM{ex}")
                nc.vector.tensor_scalar_mul(negM[:], mxs[:], -1.0)
                negMb = ex_pool.tile([128, 1], f32, tag="negMb",
                                     name=f"negMb{ex}")
                nc.gpsimd.partition_broadcast(negMb[:], negM[:])
                exp32 = ex_pool.tile([128, S // 128], f32, tag="exp",
                                     name=f"exp{ex}")
                nc.scalar.activation(exp32[:], masked[:], AF.Exp,
                                     bias=negMb[:])
                s128 = ex_pool.tile([128, 1], f32, tag="s128",
                                    name=f"s128{ex}")
                nc.vector.reduce_sum(s128[:], exp32[:], axis=AX)
                ps_sum = psS.tile([1, 1], f32, tag="psS", name=f"pssum{ex}")
                nc.tensor.matmul(ps_sum[:], s128[:], ones_sb[:], start=True,
                                 stop=True)
                rs = small_pool.tile([1, 1], f32, tag="rs", name=f"rs{ex}")
                nc.vector.reciprocal(rs[:], ps_sum[:])
                rsb = ex_pool.tile([128, 1], f32, tag="rsb", name=f"rsb{ex}")
                nc.gpsimd.partition_broadcast(rsb[:], rs[:])
                probs_t = ex_pool.tile([128, S // 128], f32r, tag="probs",
                                       name=f"probs{ex}")
                nc.vector.tensor_scalar_mul(probs_t[:], exp32[:], rsb[:])
                pPT = psS.tile([32, 128], f32r, tag="psS",
                               name=f"pPT{ex}")
                nc.tensor.transpose(pPT[:], probs_t[:], idr[:])
                probs_row = small_pool.tile([32, 128], f32r, tag="prow",
                                            name=f"prow{ex}")
                nc.vector.tensor_copy(probs_row[:], pPT[:])
                nc.sync.dma_start(
                    probs_d.ap()[ex].rearrange("(j p) -> j p", p=128),
                    probs_row[:].bitcast(f32))

                # ---- attn = probs @ enc (accumulate over 32 s-blocks) ----
                pA = psA.tile([1, 512], f32, tag="psA", name=f"pA{ex}")
                for st in range(ST):
                    for j in range(4):
                        c = 4 * st + j
                        nc.tensor.matmul(pA[:], probs_t[:, c:c + 1],
                                         chunks[st][:, 512 * j:512 * (j + 1)],
                                         start=(c == 0), stop=(c == 31))
                arow = small_pool.tile([1, 512], f32, tag="arow",
                                       name=f"arow{ex}")
                nc.vector.tensor_copy(arow[:], pA[:])
                nc.sync.dma_start(attn_d.ap()[ex:ex + 1, :], arow[:])
    nc.compile()
    _built["nc"] = nc
    return nc


def kernel(enc, dec, inp_mask, W_enc, W_dec, W_out, _trace=False):
    enc = np.ascontiguousarray(np.asarray(enc, dtype=np.float32))
    dec = np.asarray(dec, dtype=np.float32)
    inp_mask = np.asarray(inp_mask)
    W_enc = np.ascontiguousarray(np.asarray(W_enc, dtype=np.float32))
    W_dec = np.ascontiguousarray(np.asarray(W_dec, dtype=np.float32))
    W_out = np.asarray(W_out, dtype=np.float32)

    nc = _build()
    madd = (inp_mask.astype(np.float32) - 1.0) * 1.0e9          # [B, S]
    madd = np.ascontiguousarray(
        madd.reshape(B, S // 128, 128).transpose(0, 2, 1))       # [B,128,S/128]
    wout16 = np.ascontiguousarray(W_out.reshape(4, 128, 1))
    id32 = np.eye(128, dtype=np.float32)
    ones = np.ones((128, 1), dtype=np.float32)

    in_maps = []
    for c in range(N_CORES):
        b0 = c * BPC
        decT = np.ascontiguousarray(
            dec[b0:b0 + BPC].T.reshape(4, 128, BPC))
        in_maps.append({
            "enc": enc[b0:b0 + BPC],
            "decT": decT,
            "madd": madd[b0:b0 + BPC],
            "wenc": W_enc,
            "wdec": W_dec,
            "wout16": wout16,
            "id32": id32,
            "idr": id32,
            "ones": ones,
        })
    res = bass_utils.run_bass_kernel_spmd(
        nc, in_maps, core_ids=list(range(N_CORES)), trace=_trace)
    attn = np.concatenate([res.results[c]["attn_out"] for c in range(N_CORES)])
    probs = np.concatenate([res.results[c]["probs_out"]
                            for c in range(N_CORES)])
    if _trace:
        kernel._last_exec_time_ns = res.exec_time_ns
    return (attn.astype(np.float32), probs.astype(np.float32))


# revision 14
# speedup vs baseline: 1.0095x; 1.0064x over previous
"""Bahdanau additive attention on 8 Trainium2 NeuronCores.

Data-parallel over batch B=64 (8 examples/core). Per example:
  encoded^T = W_enc^T @ enc^T   (f32r matmuls; enc transposed on-chip via PE)
  result    = tanh(encoded^T + decoded^T)  (ACT, fp16 out)
  logits    = W_out^T @ result  (fp16 matmuls, 4-way tile_position col-packed)
  probs     = softmax(mask(logits))        (DVE/ACT + PE cross-partition reduce)
  attn      = probs @ enc                  (fp16 matmuls, 4-way col-packed)

Precision: main matmuls run in float32r (fp32 bits, PE rounds to ~11 mantissa
bits on read); logits/attn run fp16 -> probs/attn scale-relative error ~7e-3
vs the fp32 reference.
"""
import numpy as np

import concourse.bass as bass
import concourse.mybir as mybir
from concourse import bacc, bass_utils
from concourse.tile import TileContext

N_CORES = 8
B, S, E, H = 64, 4096, 512, 512
BPC = B // N_CORES            # examples per core
ST = S // 512                 # 8 s-tiles of 512 per example
f32 = mybir.dt.float32
f32r = mybir.dt.float32r
f16 = mybir.dt.float16
AX = mybir.AxisListType.X
AF = mybir.ActivationFunctionType

_built = {}


def _build():
    if "nc" in _built:
        return _built["nc"]
    nc = bacc.Bacc("TRN2", target_bir_lowering=False, debug=False,
                   num_devices=N_CORES)
    enc_d = nc.dram_tensor("enc", [BPC, S, E], f32r, kind="ExternalInput")
    decT_d = nc.dram_tensor("decT", [4, 128, BPC], f32, kind="ExternalInput")
    madd_d = nc.dram_tensor("madd", [BPC, 128, S // 128], f32,
                            kind="ExternalInput")
    wenc_d = nc.dram_tensor("wenc", [E, H], f32r, kind="ExternalInput")
    wdec_d = nc.dram_tensor("wdec", [E, H], f32, kind="ExternalInput")
    wout_d = nc.dram_tensor("wout16", [4, 128, 1], f16, kind="ExternalInput")
    id32_d = nc.dram_tensor("id32", [128, 128], f32, kind="ExternalInput")
    idr_d = nc.dram_tensor("idr", [128, 128], f32r, kind="ExternalInput")
    ones_d = nc.dram_tensor("ones", [128, 1], f32, kind="ExternalInput")
    attn_d = nc.dram_tensor("attn_out", [BPC, E], f32, kind="ExternalOutput")
    probs_d = nc.dram_tensor("probs_out", [BPC, S], f32, kind="ExternalOutput")

    with TileContext(nc) as tc:
        with (tc.tile_pool(name="const", bufs=1) as cpool,
              tc.tile_pool(name="encN", bufs=5) as encN_pool,
              tc.tile_pool(name="enc16", bufs=12) as enc16_pool,
              tc.tile_pool(name="encT", bufs=9) as encT_pool,
              tc.tile_pool(name="res", bufs=20) as res_pool,
              tc.tile_pool(name="small", bufs=2) as small_pool,
              tc.tile_pool(name="expool", bufs=3) as ex_pool,
              tc.tile_pool(name="psT", bufs=2, space="PSUM") as psT,
              tc.tile_pool(name="psM", bufs=3, space="PSUM") as psM,
              tc.tile_pool(name="psS", bufs=2, space="PSUM") as psS,
              tc.tile_pool(name="psA", bufs=1, space="PSUM") as psA):
            # ---- constants (dec-path + identity first so PE starts early) --
            decT_sb = []
            for db in range(4):
                t = cpool.tile([128, BPC], f32, name=f"decT{db}")
                nc.sync.dma_start(t[:], decT_d.ap()[db])
                decT_sb.append(t)
            wdec_sb = []
            for db in range(4):
                t = cpool.tile([128, H], f32, name=f"wdec{db}")
                nc.sync.dma_start(t[:], wdec_d.ap()[128 * db:128 * (db + 1), :])
                wdec_sb.append(t)
            idr = cpool.tile([128, 128], f32r, name="idrsb")
            nc.sync.dma_start(idr[:], idr_d.ap())
            id32 = cpool.tile([128, 128], f32, name="id32sb")
            nc.sync.dma_start(id32[:], id32_d.ap())
            wenc_sb = []
            for eb in range(4):
                t = cpool.tile([128, H], f32r, name=f"wenc{eb}")
                nc.sync.dma_start(t[:], wenc_d.ap()[128 * eb:128 * (eb + 1), :])
                wenc_sb.append(t)
            ones_sb = cpool.tile([128, 1], f32, name="ones_sb")
            nc.sync.dma_start(ones_sb[:], ones_d.ap())
            wout16 = cpool.tile([128, 4], f16, name="wout16sb")
            for hb in range(4):
                nc.sync.dma_start(wout16[:, hb:hb + 1], wout_d.ap()[hb])

            # ---- decoded^T = W_dec^T @ dec^T for all BPC examples ----
            dec_t = []
            for hb in range(4):
                pd = psM.tile([128, BPC], f32, tag="psM", name=f"pdec{hb}")
                for db in range(4):
                    nc.tensor.matmul(pd[:],
                                     wdec_sb[db][:, 128 * hb:128 * (hb + 1)],
                                     decT_sb[db][:], start=(db == 0),
                                     stop=(db == 3))
                t = cpool.tile([128, BPC], f32, name=f"decthb{hb}")
                nc.vector.tensor_copy(t[:], pd[:])
                dec_t.append(t)

            # ---- per-example pipeline ----
            for ex in range(BPC):
                chunks = []
                for st in range(ST):
                    ch = encN_pool.tile([128, 2048], f32r, tag="encN",
                                        name=f"encN_{ex}_{st}")
                    src = enc_d.ap()[ex].rearrange(
                        "(st j p) e -> st p j e", j=4, p=128)[st]
                    nc.sync.dma_start(
                        ch[:].rearrange("p (j e) -> p j e", j=4), src)
                    chunks.append(ch)
                madd_t = ex_pool.tile([128, S // 128], f32, tag="madd",
                                      name=f"madd{ex}")
                nc.sync.dma_start(madd_t[:], madd_d.ap()[ex])

                logits32 = ex_pool.tile([128, S // 128], f32, tag="logits",
                                        name=f"lg{ex}")
                enc16 = {}
                res16 = {}
                for sp in range(ST // 2):
                    sts = (2 * sp, 2 * sp + 1)
                    # transpose enc chunks -> encT tiles [128e, 512s]
                    encT = {}
                    for st in sts:
                        for eb in range(4):
                            pT = psT.tile([128, 512], f32r, tag="psT",
                                          name=f"pT{ex}_{st}_{eb}")
                            for j in range(4):
                                o = j * 512 + eb * 128
                                nc.tensor.transpose(
                                    pT[:, 128 * j:128 * (j + 1)],
                                    chunks[st][:, o:o + 128], idr[:])
                            eT = encT_pool.tile([128, 512], f32r, tag="encT",
                                                name=f"eT{ex}_{st}_{eb}")
                            if eb % 2 == 0:
                                nc.vector.tensor_copy(eT[:], pT[:])
                            else:
                                nc.scalar.copy(eT[:], pT[:])
                            encT[(st, eb)] = eT
                    # encoded^T + tanh -> fp16 result tiles [128h, 512s]
                    for hb in range(4):
                        pM = {st: psM.tile([128, 512], f32, tag="psM",
                                           name=f"pM{ex}_{st}_{hb}")
                              for st in sts}
                        for eb in range(4):
                            for st in sts:
                                nc.tensor.matmul(
                                    pM[st][:],
                                    wenc_sb[eb][:, 128 * hb:128 * (hb + 1)],
                                    encT[(st, eb)][:], start=(eb == 0),
                                    stop=(eb == 3))
                        for st in sts:
                            r = res_pool.tile([128, 512], f16, tag="res",
                                              name=f"res{ex}_{st}_{hb}")
                            nc.scalar.activation(r[:], pM[st][:], AF.Tanh,
                                                 bias=dec_t[hb][:, ex:ex + 1])
                            res16[(st, hb)] = r
                    # fp16 copies of the natural chunks (for packed attn);
                    # lets the f32r chunk slot recycle after transposes
                    for st in sts:
                        c16 = enc16_pool.tile([128, 2048], f16, tag="enc16",
                                              name=f"enc16_{ex}_{st}")
                        nc.vector.tensor_copy(c16[:], chunks[st][:].bitcast(f32))
                        enc16[st] = c16

                    if sp % 2 == 1:
                        # ---- logits for s-tiles 4P..4P+3, col-packed ----
                        P = sp // 2
                        pL4 = psS.tile([128, 512], f32, tag="psS",
                                       name=f"pL4_{ex}_{P}")
                        for hb in range(4):
                            for g in range(4):
                                st = 4 * P + g
                                nc.tensor.matmul(
                                    pL4[32 * g:32 * g + 1, :],
                                    wout16[:, hb:hb + 1],
                                    res16[(st, hb)][:], start=(hb == 0),
                                    stop=(hb == 3),
                                    tile_position=(0, 32 * g))
                        sbL = small_pool.tile([128, 512], f32, tag="sbL",
                                              name=f"sbL{ex}_{P}")
                        nc.vector.tensor_copy(sbL[:], pL4[:])
                        pLT = psS.tile([128, 512], f32, tag="psS",
                                       name=f"pLT{ex}_{P}")
                        for c in range(4):
                            nc.tensor.transpose(
                                pLT[:, 128 * c:128 * (c + 1)],
                                sbL[:, 128 * c:128 * (c + 1)], id32[:])
                        sbLT = small_pool.tile([128, 512], f32, tag="sbLT",
                                               name=f"sbLT{ex}_{P}")
                        nc.vector.tensor_copy(sbLT[:], pLT[:])
                        for c in range(4):
                            nc.vector.tensor_copy(
                                logits32[:, 16 * P + c:16 * P + c + 13:4],
                                sbLT[:, 128 * c:128 * c + 97:32])

                # ---- softmax over the 4096 logits ----
                masked = ex_pool.tile([128, S // 128], f32, tag="masked",
                                      name=f"msk{ex}")
                nc.vector.tensor_add(masked[:], logits32[:], madd_t[:])
                mx128 = ex_pool.tile([128, 1], f32, tag="mx", name=f"mx{ex}")
                nc.vector.reduce_max(mx128[:], masked[:], axis=AX)
                pmx = psS.tile([1, 128], f32, tag="psS", name=f"pmx{ex}")
                nc.tensor.transpose(pmx[:], mx128[:], id32[:])
                mxrow = small_pool.tile([1, 128], f32, tag="mxrow",
                                        name=f"mxrow{ex}")
                nc.vector.tensor_copy(mxrow[:], pmx[:])
                mxs = small_pool.tile([1, 1], f32, tag="mxs", name=f"mxs{ex}")
                nc.vector.reduce_max(mxs[:], mxrow[:], axis=AX)
                negM = small_pool.tile([1, 1], f32, tag="negM",
                                       name=f"negM{ex}")
                nc.vector.tensor_scalar_mul(negM[:], mxs[:], -1.0)
                negMb = ex_pool.tile([128, 1], f32, tag="negMb",
                                     name=f"negMb{ex}")
                nc.gpsimd.partition_broadcast(negMb[:], negM[:])
                exp32 = ex_pool.tile([128, S // 128], f32, tag="exp",
                                     name=f"exp{ex}")
                nc.scalar.activation(exp32[:], masked[:], AF.Exp,
                                     bias=negMb[:])
                s128 = ex_pool.tile([128, 1], f32, tag="s128",
                                    name=f"s128{ex}")
                nc.vector.reduce_sum(s128[:], exp32[:], axis=AX)
                ps_sum = psS.tile([1, 1], f32, tag="psS", name=f"pssum{ex}")
                nc.tensor.matmul(ps_sum[:], s128[:], ones_sb[:], start=True,
                                 stop=True)
                rs = small_pool.tile([1, 1], f32, tag="rs", name=f"rs{ex}")
                nc.vector.reciprocal(rs[:], ps_sum[:])
                rsb = ex_pool.tile([128, 1], f32, tag="rsb", name=f"rsb{ex}")
                nc.gpsimd.partition_broadcast(rsb[:], rs[:])
                probs_t = ex_pool.tile([128, S // 128], f32r, tag="probs",
                                       name=f"probs{ex}")
                nc.vector.tensor_scalar_mul(probs_t[:], exp32[:], rsb[:])
                probs16 = ex_pool.tile([128, S // 128], f16, tag="probs16",
                                       name=f"probs16_{ex}")
                nc.vector.tensor_copy(probs16[:], probs_t[:].bitcast(f32))
                pPT = psS.tile([32, 128], f32r, tag="psS", name=f"pPT{ex}")
                nc.tensor.transpose(pPT[:], probs_t[:], idr[:])
                probs_row = small_pool.tile([32, 128], f32r, tag="prow",
                                            name=f"prow{ex}")
                nc.vector.tensor_copy(probs_row[:], pPT[:])
                nc.sync.dma_start(
                    probs_d.ap()[ex].rearrange("(j p) -> j p", p=128),
                    probs_row[:].bitcast(f32))

                # ---- attn = probs @ enc, 4-way col-packed ----
                pP = psA.tile([128, 512], f32, tag="psA", name=f"pP{ex}")
                for r in range(8):
                    for g in range(4):
                        c = 4 * r + g
                        nc.tensor.matmul(
                            pP[32 * g:32 * g + 1, :],
                            probs16[:, c:c + 1],
                            enc16[r][:, 512 * g:512 * (g + 1)],
                            start=(r == 0), stop=(r == 7),
                            tile_position=(0, 32 * g))
                sb4 = small_pool.tile([128, 512], f32, tag="sb4",
                                      name=f"sb4_{ex}")
                nc.vector.tensor_copy(sb4[:], pP[:])
                pTA = psA.tile([128, 512], f32, tag="psA", name=f"pTA{ex}")
                for c in range(4):
                    nc.tensor.transpose(pTA[:, 128 * c:128 * (c + 1)],
                                        sb4[:, 128 * c:128 * (c + 1)],
                                        id32[:])
                sbT = small_pool.tile([128, 512], f32, tag="sbT",
                                      name=f"sbT{ex}")
                nc.vector.tensor_copy(sbT[:], pTA[:])
                attn4 = small_pool.tile([128, 4], f32, tag="attn4",
                                        name=f"attn4_{ex}")
                for c in range(4):
                    nc.vector.reduce_sum(attn4[:, c:c + 1],
                                         sbT[:, 128 * c:128 * c + 97:32],
                                         axis=AX)
                nc.sync.dma_start(
                    attn_d.ap()[ex].rearrange("(c p) -> p c", p=128),
                    attn4[:])
    nc.compile()
    _built["nc"] = nc
    return nc


def kernel(enc, dec, inp_mask, W_enc, W_dec, W_out, _trace=False):
    enc = np.ascontiguousarray(np.asarray(enc, dtype=np.float32))
    dec = np.asarray(dec, dtype=np.float32)
    inp_mask = np.asarray(inp_mask)
    W_enc = np.ascontiguousarray(np.asarray(W_enc, dtype=np.float32))
    W_dec = np.ascontiguousarray(np.asarray(W_dec, dtype=np.float32))
    W_out = np.asarray(W_out, dtype=np.float32)

    nc = _build()
    madd = (inp_mask.astype(np.float32) - 1.0) * 1.0e9          # [B, S]
    madd = np.ascontiguousarray(
        madd.reshape(B, S // 128, 128).transpose(0, 2, 1))       # [B,128,S/128]
    wout16 = np.ascontiguousarray(
        W_out.reshape(4, 128, 1).astype(np.float16))
    id32 = np.eye(128, dtype=np.float32)
    ones = np.ones((128, 1), dtype=np.float32)

    in_maps = []
    for c in range(N_CORES):
        b0 = c * BPC
        decT = np.ascontiguousarray(
            dec[b0:b0 + BPC].T.reshape(4, 128, BPC))
        in_maps.append({
            "enc": enc[b0:b0 + BPC],
            "decT": decT,
            "madd": madd[b0:b0 + BPC],
            "wenc": W_enc,
            "wdec": W_dec,
            "wout16": wout16,
            "id32": id32,
            "idr": id32,
            "ones": ones,
        })
    res = bass_utils.run_bass_kernel_spmd(
        nc, in_maps, core_ids=list(range(N_CORES)), trace=_trace)
    attn = np.concatenate([res.results[c]["attn_out"] for c in range(N_CORES)])
    probs = np.concatenate([res.results[c]["probs_out"]
                            for c in range(N_CORES)])
    if _trace:
        kernel._last_exec_time_ns = res.exec_time_ns
    return (attn.astype(np.float32), probs.astype(np.float32))


# revision 15
# speedup vs baseline: 1.0801x; 1.0699x over previous
"""Bahdanau additive attention on 8 Trainium2 NeuronCores.

Data-parallel over batch B=64 (8 examples/core). Per example:
  encoded^T = W_enc^T @ enc^T   (f32r matmuls; enc transposed on-chip via PE)
  result    = tanh(encoded^T + decoded^T)  (ACT, fp16 out)
  logits    = W_out^T @ result  (fp16 matmuls, 4-way tile_position col-packed)
  probs     = softmax(mask(logits))        (DVE/ACT + PE cross-partition reduce)
  attn      = probs @ enc                  (fp16 matmuls, 4-way col-packed)

Precision: main matmuls run in float32r (fp32 bits, PE rounds to ~11 mantissa
bits on read); logits/attn run fp16 -> probs/attn scale-relative error ~7e-3
vs the fp32 reference.
"""
import numpy as np

import concourse.bass as bass
import concourse.mybir as mybir
from concourse import bacc, bass_utils
from concourse.tile import TileContext

N_CORES = 8
B, S, E, H = 64, 4096, 512, 512
BPC = B // N_CORES            # examples per core
ST = S // 512                 # 8 s-tiles of 512 per example
f32 = mybir.dt.float32
f32r = mybir.dt.float32r
f16 = mybir.dt.float16
AX = mybir.AxisListType.X
AF = mybir.ActivationFunctionType

_built = {}


def _build():
    if "nc" in _built:
        return _built["nc"]
    nc = bacc.Bacc("TRN2", target_bir_lowering=False, debug=False,
                   num_devices=N_CORES)
    enc_d = nc.dram_tensor("enc", [BPC, S, E], f32r, kind="ExternalInput")
    decT_d = nc.dram_tensor("decT", [4, 128, BPC], f32, kind="ExternalInput")
    madd_d = nc.dram_tensor("madd", [BPC, 128, S // 128], f32,
                            kind="ExternalInput")
    wenc_d = nc.dram_tensor("wenc", [E, H], f32r, kind="ExternalInput")
    wdec_d = nc.dram_tensor("wdec", [E, H], f32, kind="ExternalInput")
    wout_d = nc.dram_tensor("wout16", [4, 128, 1], f16, kind="ExternalInput")
    id32_d = nc.dram_tensor("id32", [128, 128], f32, kind="ExternalInput")
    idr_d = nc.dram_tensor("idr", [128, 128], f32r, kind="ExternalInput")
    ones_d = nc.dram_tensor("ones", [128, 1], f32, kind="ExternalInput")
    attn_d = nc.dram_tensor("attn_out", [BPC, E], f32, kind="ExternalOutput")
    probs_d = nc.dram_tensor("probs_out", [BPC, S], f32, kind="ExternalOutput")

    with TileContext(nc) as tc:
        with (tc.tile_pool(name="const", bufs=1) as cpool,
              tc.tile_pool(name="encN", bufs=8) as encN_pool,
              tc.tile_pool(name="enc16", bufs=10) as enc16_pool,
              tc.tile_pool(name="encT", bufs=9) as encT_pool,
              tc.tile_pool(name="res", bufs=18) as res_pool,
              tc.tile_pool(name="small", bufs=2) as small_pool,
              tc.tile_pool(name="expool", bufs=3) as ex_pool,
              tc.tile_pool(name="psT", bufs=2, space="PSUM") as psT,
              tc.tile_pool(name="psM", bufs=3, space="PSUM") as psM,
              tc.tile_pool(name="psS", bufs=2, space="PSUM") as psS,
              tc.tile_pool(name="psA", bufs=1, space="PSUM") as psA):
            # ---- constants (dec-path + identity first so PE starts early) --
            decT_sb = []
            for db in range(4):
                t = cpool.tile([128, BPC], f32, name=f"decT{db}")
                nc.sync.dma_start(t[:], decT_d.ap()[db])
                decT_sb.append(t)
            wdec_sb = []
            for db in range(4):
                t = cpool.tile([128, H], f32, name=f"wdec{db}")
                nc.sync.dma_start(t[:], wdec_d.ap()[128 * db:128 * (db + 1), :])
                wdec_sb.append(t)
            idr = cpool.tile([128, 128], f32r, name="idrsb")
            nc.sync.dma_start(idr[:], idr_d.ap())
            id32 = cpool.tile([128, 128], f32, name="id32sb")
            nc.sync.dma_start(id32[:], id32_d.ap())
            wenc_sb = []
            for eb in range(4):
                t = cpool.tile([128, H], f32r, name=f"wenc{eb}")
                nc.sync.dma_start(t[:], wenc_d.ap()[128 * eb:128 * (eb + 1), :])
                wenc_sb.append(t)
            ones_sb = cpool.tile([128, 1], f32, name="ones_sb")
            nc.sync.dma_start(ones_sb[:], ones_d.ap())
            wout16 = cpool.tile([128, 4], f16, name="wout16sb")
            for hb in range(4):
                nc.sync.dma_start(wout16[:, hb:hb + 1], wout_d.ap()[hb])

            # ---- decoded^T = W_dec^T @ dec^T for all BPC examples ----
            dec_t = []
            for hb in range(4):
                pd = psM.tile([128, BPC], f32, tag="psM", name=f"pdec{hb}")
                for db in range(4):
                    nc.tensor.matmul(pd[:],
                                     wdec_sb[db][:, 128 * hb:128 * (hb + 1)],
                                     decT_sb[db][:], start=(db == 0),
                                     stop=(db == 3))
                t = cpool.tile([128, BPC], f32, name=f"decthb{hb}")
                nc.vector.tensor_copy(t[:], pd[:])
                dec_t.append(t)

            # ---- per-example pipeline ----
            for ex in range(BPC):
                chunks = []
                for st in range(ST):
                    ch = encN_pool.tile([128, 2048], f32r, tag="encN",
                                        name=f"encN_{ex}_{st}")
                    src = enc_d.ap()[ex].rearrange(
                        "(st j p) e -> st p j e", j=4, p=128)[st]
                    nc.sync.dma_start(
                        ch[:].rearrange("p (j e) -> p j e", j=4), src)
                    chunks.append(ch)
                madd_t = ex_pool.tile([128, S // 128], f32, tag="madd",
                                      name=f"madd{ex}")
                nc.sync.dma_start(madd_t[:], madd_d.ap()[ex])

                logits32 = ex_pool.tile([128, S // 128], f32, tag="logits",
                                        name=f"lg{ex}")
                enc16 = {}
                res16 = {}
                pending = []
                for sp in range(ST // 2):
                    sts = (2 * sp, 2 * sp + 1)
                    # transpose enc chunks -> encT tiles [128e, 512s]
                    encT = {}
                    for st in sts:
                        for eb in range(4):
                            pT = psT.tile([128, 512], f32r, tag="psT",
                                          name=f"pT{ex}_{st}_{eb}")
                            for j in range(4):
                                o = j * 512 + eb * 128
                                nc.tensor.transpose(
                                    pT[:, 128 * j:128 * (j + 1)],
                                    chunks[st][:, o:o + 128], idr[:])
                            eT = encT_pool.tile([128, 512], f32r, tag="encT",
                                                name=f"eT{ex}_{st}_{eb}")
                            if eb % 2 == 0:
                                nc.vector.tensor_copy(eT[:], pT[:])
                            else:
                                nc.scalar.copy(eT[:], pT[:])
                            encT[(st, eb)] = eT
                    # encoded^T + tanh -> fp16 result tiles [128h, 512s]
                    for hb in range(4):
                        pM = {st: psM.tile([128, 512], f32, tag="psM",
                                           name=f"pM{ex}_{st}_{hb}")
                              for st in sts}
                        for eb in range(4):
                            for st in sts:
                                nc.tensor.matmul(
                                    pM[st][:],
                                    wenc_sb[eb][:, 128 * hb:128 * (hb + 1)],
                                    encT[(st, eb)][:], start=(eb == 0),
                                    stop=(eb == 3))
                        for st in sts:
                            r = res_pool.tile([128, 512], f16, tag="res",
                                              name=f"res{ex}_{st}_{hb}")
                            nc.scalar.activation(r[:], pM[st][:], AF.Tanh,
                                                 bias=dec_t[hb][:, ex:ex + 1])
                            res16[(st, hb)] = r
                    # fp16 copies of the natural chunks (for packed attn),
                    # deferred one pair so they sit behind the next pair's
                    # critical encT copies in the DVE queue
                    for st in pending:
                        c16 = enc16_pool.tile([128, 2048], f16, tag="enc16",
                                              name=f"enc16_{ex}_{st}")
                        nc.vector.tensor_copy(c16[:], chunks[st][:].bitcast(f32))
                        enc16[st] = c16
                    pending = list(sts)

                    if sp % 2 == 1:
                        # ---- logits for s-tiles 4P..4P+3, col-packed ----
                        P = sp // 2
                        pL4 = psS.tile([128, 512], f32, tag="psS",
                                       name=f"pL4_{ex}_{P}")
                        for hb in range(4):
                            for g in range(4):
                                st = 4 * P + g
                                nc.tensor.matmul(
                                    pL4[32 * g:32 * g + 1, :],
                                    wout16[:, hb:hb + 1],
                                    res16[(st, hb)][:], start=(hb == 0),
                                    stop=(hb == 3),
                                    tile_position=(0, 32 * g))
                        sbL = small_pool.tile([128, 512], f32, tag="sbL",
                                              name=f"sbL{ex}_{P}")
                        nc.scalar.copy(sbL[:], pL4[:])
                        pLT = psS.tile([128, 512], f32, tag="psS",
                                       name=f"pLT{ex}_{P}")
                        for c in range(4):
                            nc.tensor.transpose(
                                pLT[:, 128 * c:128 * (c + 1)],
                                sbL[:, 128 * c:128 * (c + 1)], id32[:])
                        sbLT = small_pool.tile([128, 512], f32, tag="sbLT",
                                               name=f"sbLT{ex}_{P}")
                        nc.scalar.copy(sbLT[:], pLT[:])
                        for c in range(4):
                            nc.vector.tensor_copy(
                                logits32[:, 16 * P + c:16 * P + c + 13:4],
                                sbLT[:, 128 * c:128 * c + 97:32])

                for st in pending:
                    c16 = enc16_pool.tile([128, 2048], f16, tag="enc16",
                                          name=f"enc16_{ex}_{st}")
                    nc.vector.tensor_copy(c16[:], chunks[st][:].bitcast(f32))
                    enc16[st] = c16

                # ---- softmax over the 4096 logits ----
                masked = ex_pool.tile([128, S // 128], f32, tag="masked",
                                      name=f"msk{ex}")
                nc.vector.tensor_add(masked[:], logits32[:], madd_t[:])
                mx128 = ex_pool.tile([128, 1], f32, tag="mx", name=f"mx{ex}")
                nc.vector.reduce_max(mx128[:], masked[:], axis=AX)
                pmx = psS.tile([1, 128], f32, tag="psS", name=f"pmx{ex}")
                nc.tensor.transpose(pmx[:], mx128[:], id32[:])
                mxrow = small_pool.tile([1, 128], f32, tag="mxrow",
                                        name=f"mxrow{ex}")
                nc.vector.tensor_copy(mxrow[:], pmx[:])
                mxs = small_pool.tile([1, 1], f32, tag="mxs", name=f"mxs{ex}")
                nc.vector.reduce_max(mxs[:], mxrow[:], axis=AX)
                negM = small_pool.tile([1, 1], f32, tag="negM",
                                       name=f"negM{ex}")
                nc.vector.tensor_scalar_mul(negM[:], mxs[:], -1.0)
                negMb = ex_pool.tile([128, 1], f32, tag="negMb",
                                     name=f"negMb{ex}")
                nc.gpsimd.partition_broadcast(negMb[:], negM[:])
                exp32 = ex_pool.tile([128, S // 128], f32, tag="exp",
                                     name=f"exp{ex}")
                nc.scalar.activation(exp32[:], masked[:], AF.Exp,
                                     bias=negMb[:])
                s128 = ex_pool.tile([128, 1], f32, tag="s128",
                                    name=f"s128{ex}")
                nc.vector.reduce_sum(s128[:], exp32[:], axis=AX)
                ps_sum = psS.tile([1, 1], f32, tag="psS", name=f"pssum{ex}")
                nc.tensor.matmul(ps_sum[:], s128[:], ones_sb[:], start=True,
                                 stop=True)
                rs = small_pool.tile([1, 1], f32, tag="rs", name=f"rs{ex}")
                nc.vector.reciprocal(rs[:], ps_sum[:])
                rsb = ex_pool.tile([128, 1], f32, tag="rsb", name=f"rsb{ex}")
                nc.gpsimd.partition_broadcast(rsb[:], rs[:])
                probs_t = ex_pool.tile([128, S // 128], f32r, tag="probs",
                                       name=f"probs{ex}")
                nc.vector.tensor_scalar_mul(probs_t[:], exp32[:], rsb[:])
                probs16 = ex_pool.tile([128, S // 128], f16, tag="probs16",
                                       name=f"probs16_{ex}")
                nc.vector.tensor_copy(probs16[:], probs_t[:].bitcast(f32))
                pPT = psS.tile([32, 128], f32r, tag="psS", name=f"pPT{ex}")
                nc.tensor.transpose(pPT[:], probs_t[:], idr[:])
                probs_row = small_pool.tile([32, 128], f32r, tag="prow",
                                            name=f"prow{ex}")
                nc.vector.tensor_copy(probs_row[:], pPT[:])
                nc.sync.dma_start(
                    probs_d.ap()[ex].rearrange("(j p) -> j p", p=128),
                    probs_row[:].bitcast(f32))

                # ---- attn = probs @ enc, 4-way col-packed ----
                pP = psA.tile([128, 512], f32, tag="psA", name=f"pP{ex}")
                for r in range(8):
                    for g in range(4):
                        c = 4 * r + g
                        nc.tensor.matmul(
                            pP[32 * g:32 * g + 1, :],
                            probs16[:, c:c + 1],
                            enc16[r][:, 512 * g:512 * (g + 1)],
                            start=(r == 0), stop=(r == 7),
                            tile_position=(0, 32 * g))
                sb4 = small_pool.tile([128, 512], f32, tag="sb4",
                                      name=f"sb4_{ex}")
                nc.scalar.copy(sb4[:], pP[:])
                pTA = psA.tile([128, 512], f32, tag="psA", name=f"pTA{ex}")
                for c in range(4):
                    nc.tensor.transpose(pTA[:, 128 * c:128 * (c + 1)],
                                        sb4[:, 128 * c:128 * (c + 1)],
                                        id32[:])
                sbT = small_pool.tile([128, 512], f32, tag="sbT",
                                      name=f"sbT{ex}")
                nc.scalar.copy(sbT[:], pTA[:])
                attn4 = small_pool.tile([128, 4], f32, tag="attn4",
                                        name=f"attn4_{ex}")
                for c in range(4):
                    nc.vector.reduce_sum(attn4[:, c:c + 1],
                                         sbT[:, 128 * c:128 * c + 97:32],
                                         axis=AX)
                nc.sync.dma_start(
                    attn_d.ap()[ex].rearrange("(c p) -> p c", p=128),
                    attn4[:])
    nc.compile()
    _built["nc"] = nc
    return nc


def kernel(enc, dec, inp_mask, W_enc, W_dec, W_out, _trace=False):
    enc = np.ascontiguousarray(np.asarray(enc, dtype=np.float32))
    dec = np.asarray(dec, dtype=np.float32)
    inp_mask = np.asarray(inp_mask)
    W_enc = np.ascontiguousarray(np.asarray(W_enc, dtype=np.float32))
    W_dec = np.ascontiguousarray(np.asarray(W_dec, dtype=np.float32))
    W_out = np.asarray(W_out, dtype=np.float32)

    nc = _build()
    madd = (inp_mask.astype(np.float32) - 1.0) * 1.0e9          # [B, S]
    madd = np.ascontiguousarray(
        madd.reshape(B, S // 128, 128).transpose(0, 2, 1))       # [B,128,S/128]
    wout16 = np.ascontiguousarray(
        W_out.reshape(4, 128, 1).astype(np.float16))
    id32 = np.eye(128, dtype=np.float32)
    ones = np.ones((128, 1), dtype=np.float32)

    in_maps = []
    for c in range(N_CORES):
        b0 = c * BPC
        decT = np.ascontiguousarray(
            dec[b0:b0 + BPC].T.reshape(4, 128, BPC))
        in_maps.append({
            "enc": enc[b0:b0 + BPC],
            "decT": decT,
            "madd": madd[b0:b0 + BPC],
            "wenc": W_enc,
            "wdec": W_dec,
            "wout16": wout16,
            "id32": id32,
            "idr": id32,
            "ones": ones,
        })
    res = bass_utils.run_bass_kernel_spmd(
        nc, in_maps, core_ids=list(range(N_CORES)), trace=_trace)
    attn = np.concatenate([res.results[c]["attn_out"] for c in range(N_CORES)])
    probs = np.concatenate([res.results[c]["probs_out"]
                            for c in range(N_CORES)])
    if _trace:
        kernel._last_exec_time_ns = res.exec_time_ns
    return (attn.astype(np.float32), probs.astype(np.float32))
